# revision 20
# baseline (speedup 1.0000x reference)
"""Trainium2 Bass kernel for nn_Attention_5093831213465 (v2).

Reference computation (per sample, x_b: [256, 4096]):
  q = Wq @ x_b                       [32, 4096]
  k = maxpool2(Wk @ x_b)             [32, 1024]
  v = maxpool2(Wv @ x_b)             [128, 1024]
  attn = softmax_over_k(k^T @ q)     [1024, 4096]
  out  = Wa @ (v @ attn)             [256, 4096]
  y    = gamma * out + x_b
Sharding: data-parallel over batch, 2 samples per core on 8 cores.

v2 design (cost-model driven; matmul cost = N*pe_cycle*cpr, fp8 DR=0.5):
- 1x1 convs as fp8e4 DoubleRow matmuls (host pre-splits x and weights
  into contraction pairs [K, 2, N]); one M=64 matmul yields q (psum
  rows 0-31) and k (rows 32-63) per 512-col chunk.
- attention matmuls in f32r (K=32): fp8-DR would need a 16-partition
  pair layout whose psum evacuation costs more than the PE it saves.
- exp on ACT, psum->SBUF bf16 (logits span +-28, fp8 can't hold E).
  ACT exp is ~64us; all other engines are balanced just under it.
- softmax denominator: E-tile column slices as stationary operands
  against a ones column give s^T [128q, 1] at N=1 (~zero PE engine
  time); recip runs on the transposed [128, 4] layout; a small PE
  transpose (via a stride-32 spread so rows land on partitions
  0/32/64/96) turns 1/s back into rows; partition_broadcast (Pool)
  expands them; GPSIMD cannot touch PSUM so the row tile is staged
  through SBUF by ACT.
- U (v@attn) in bf16; out = Wa@un in f32r; y = po + x_bf16 on DVE,
  stored bf16 (host converts to f32).
- per-chunk q copies alternate DVE/ACT; sample-1 conv is interleaved
  into sample-0 attention chunks so the exp stream never starves.
"""

import sys

import numpy as np

if "/opt/trn_rl_repo" not in sys.path:
    sys.path.insert(0, "/opt/trn_rl_repo")

B, C, H, W = 16, 256, 64, 64
CA = C // 8          # 32  attn channels
CS = C // 2          # 128 value channels
HWF = H * W          # 4096 spatial positions
HWP = HWF // 4       # 1024 pooled positions
SPC = 2              # samples per core
NCORES = 8
CHUNK = 512          # chunk (q columns)
NCHUNK = HWF // CHUNK       # 8
KT = HWP // 128             # 8 kk tiles of 128
KG = 2                      # kk tiles per exp group
NG = KT // KG               # 4 groups

_built = {}


def _build_program():
    from collections import deque
    from contextlib import ExitStack

    import concourse.bass as bass
    import concourse.tile as tile
    from concourse import bacc, mybir

    f32 = mybir.dt.float32
    f32r = mybir.dt.float32r
    bf16 = mybir.dt.bfloat16
    fp8 = mybir.dt.float8e4
    Exp = mybir.ActivationFunctionType.Exp
    DR = mybir.MatmulPerfMode.DoubleRow

    nc = bacc.Bacc(
        "TRN2", target_bir_lowering=False, debug=False, enable_asserts=False
    )

    x8_d = nc.dram_tensor("x8", [SPC, 128, 2, HWF], fp8, kind="ExternalInput").ap()
    xr_d = nc.dram_tensor("xr", [SPC, 2, 128, HWF], bf16, kind="ExternalInput").ap()
    wqk_d = nc.dram_tensor("wqk8", [128, 2, 64], bf16, kind="ExternalInput").ap()
    wv_d = nc.dram_tensor("wv8", [128, 2, CS], fp8, kind="ExternalInput").ap()
    wa_d = nc.dram_tensor("waTg", [CS, 2, 128], f32r, kind="ExternalInput").ap()
    id_d = nc.dram_tensor("identb", [128, 128], bf16, kind="ExternalInput").ap()
    idf_d = nc.dram_tensor("identf", [128, 128], f32, kind="ExternalInput").ap()
    on_d = nc.dram_tensor("onesb", [128, 1], bf16, kind="ExternalInput").ap()
    y_d = nc.dram_tensor("y", [SPC, C, HWF], bf16, kind="ExternalOutput").ap()

    with tile.TileContext(nc) as tc, ExitStack() as ctx:
        consts = ctx.enter_context(tc.tile_pool(name="consts", bufs=1))
        xp = ctx.enter_context(tc.tile_pool(name="xp", bufs=1))
        qcp = ctx.enter_context(tc.tile_pool(name="qcp", bufs=1))
        kvp = ctx.enter_context(tc.tile_pool(name="kvp", bufs=1))
        ep = ctx.enter_context(tc.tile_pool(name="ep", bufs=8))
        sp = ctx.enter_context(tc.tile_pool(name="sp", bufs=4))
        rp = ctx.enter_context(tc.tile_pool(name="rp", bufs=1))
        yp = ctx.enter_context(tc.tile_pool(name="yp", bufs=4))
        psQ = ctx.enter_context(tc.tile_pool(name="psQ", bufs=1, space="PSUM"))
        psA = ctx.enter_context(tc.tile_pool(name="psA", bufs=2, space="PSUM"))
        psUO = ctx.enter_context(tc.tile_pool(name="psUO", bufs=2, space="PSUM"))
        psM = ctx.enter_context(tc.tile_pool(name="psM", bufs=1, space="PSUM"))

        wqk = consts.tile([128, 2, 64], bf16)
        nc.sync.dma_start(wqk[:], wqk_d)
        wv = consts.tile([128, 2, CS], fp8)
        nc.sync.dma_start(wv[:], wv_d)
        wa = consts.tile([128, 2, 128], f32r)
        nc.sync.dma_start(wa[:], wa_d)
        ident = consts.tile([128, 128], bf16)
        nc.sync.dma_start(ident[:], id_d)
        identf = consts.tile([128, 128], f32)
        nc.sync.dma_start(identf[:], idf_d)
        ones = consts.tile([128, 1], bf16)
        nc.sync.dma_start(ones[:], on_d)

        # full-input loads, s0 first so conv(s0) starts early
        x8 = xp.tile([128, SPC, 2, HWF], fp8, name="x8", tag="x8")
        xr = xp.tile([128, SPC, 2, HWF], bf16, name="xr", tag="xr")
        for s in range(SPC):
            nc.sync.dma_start(x8[:, s, :, :], x8_d[s])
            nc.sync.dma_start(
                xr[:, s, :, :], xr_d[s].rearrange("t p f -> p t f")
            )

        # per-sample persistent SBUF
        qcf = [[None] * NCHUNK for _ in range(SPC)]
        kph = [[None] * NG for _ in range(SPC)]
        vph = [None] * SPC
        vTh = [None] * SPC
        for s in range(SPC):
            for ck in range(NCHUNK):
                qcf[s][ck] = qcp.tile(
                    [32, CHUNK], f32r, name=f"qc{s}_{ck}", tag=f"qc{s}_{ck}"
                )
            for g in range(NG):
                kph[s][g] = kvp.tile(
                    [32, KG, 128], f32r, name=f"kp{s}_{g}", tag=f"kp{s}_{g}"
                )
            vph[s] = kvp.tile([128, KT, 128], bf16, name=f"vp{s}", tag=f"vp{s}")
            vTh[s] = kvp.tile([128, KT, 128], bf16, name=f"vT{s}", tag=f"vT{s}")

        def conv_qk(s, cc):
            """qk conv chunk cc (512 cols): one DR matmul -> pq[64, 512]
            (q rows 0-31, k rows 32-63), q copy (DVE/ACT alternating),
            pooled k to kph (DVE)."""
            cs = slice(cc * CHUNK, (cc + 1) * CHUNK)
            pq = psQ.tile([64, CHUNK], f32, name="pq", tag="q")
            for t in range(2):
                nc.tensor.matmul(
                    pq[:], wqk[:, t, :], xr[:, s, t, cs],
                    start=(t == 0), stop=(t == 1),
                )
            if s == 0:
                nc.scalar.copy(qcf[s][cc][:], pq[0:32, :])
            else:
                nc.vector.tensor_copy(qcf[s][cc][:], pq[0:32, :])

            # maxpool k: 512 cols = 8 image rows -> 4 pooled rows = kt cc
            kin = pq[32:64, :].rearrange(
                "p (h2 dh w2 dw) -> p h2 w2 dh dw", h2=4, dh=2, w2=32, dw=2
            )
            nc.vector.tensor_reduce(
                kph[s][cc // KG][:, cc % KG, :].rearrange(
                    "p (h2 w) -> p h2 w", h2=4
                ),
                kin,
                axis=mybir.AxisListType.XY,
                op=mybir.AluOpType.max,
            )

        def conv_v(s, vc):
            """v conv chunk vc (512 cols): 1 DR matmul + maxpool -> vph."""
            cs = slice(vc * CHUNK, (vc + 1) * CHUNK)
            pv = psM.tile([128, CHUNK], f32, name="pv", tag="m")
            nc.tensor.matmul(
                pv[:], wv[:], x8[:, s, :, cs], start=True, stop=True,
                perf_mode=DR,
            )
            vin = pv[:].rearrange(
                "p (h2 dh w2 dw) -> p h2 w2 dh dw", h2=4, dh=2, w2=32, dw=2
            )
            nc.vector.tensor_reduce(
                vph[s][:, vc, :].rearrange("p (h2 w) -> p h2 w", h2=4),
                vin,
                axis=mybir.AxisListType.XY,
                op=mybir.AluOpType.max,
            )

        def transpose_v(s, g):
            ptr = psM.tile([128, 4, 128], bf16, name="ptr", tag="m")
            for j in range(4):
                kt = g * 4 + j
                nc.tensor.transpose(ptr[:, j, :], vph[s][:, kt, :], ident[:])
            nc.vector.tensor_copy(vTh[s][:, g * 4 : (g + 1) * 4, :], ptr[:])

        def attn_chunk(s, ck, bg, per_group=0, tail=0):
            """full attention for chunk ck; bg is a deque of conv thunks
            popped between groups to fill engine gaps."""

            def pop(n):
                for _ in range(n):
                    if bg:
                        bg.popleft()()

            cs = slice(ck * CHUNK, (ck + 1) * CHUNK)
            egs = []
            for g in range(NG):
                pa = psA.tile([128, KG, CHUNK], f32, name="pa", tag="attn")
                for j in range(KG):
                    nc.tensor.matmul(
                        pa[:, j, :],
                        kph[s][g][:, j, :],
                        qcf[s][ck][:],
                        start=True,
                        stop=True,
                    )
                eg = ep.tile([128, KG, CHUNK], bf16, name="eg", tag="E")
                nc.scalar.activation(eg[:], pa[:], Exp)
                egs.append(eg)
                pop(per_group)
            pop(tail)

            # fold 8 E tiles to 4 (one DVE add, one Pool add), then
            # transposed softmax sums: s^T[q, 1] per 128-col subtile via
            # t-slices as stationary against a ones column (N=1, ~free).
            # One global start: its pending-zero marking covers the whole
            # 2KB zero-region, so later columns' first accumulates read 0.
            t1 = sp.tile([128, KG, CHUNK], bf16, name="t1", tag="t1")
            nc.vector.tensor_add(t1[:], egs[0][:], egs[1][:])
            t2 = sp.tile([128, KG, CHUNK], bf16, name="t2", tag="t2")
            nc.gpsimd.tensor_add(t2[:], egs[2][:], egs[3][:])
            psm = psM.tile([1, CHUNK], f32, name="psm", tag="m")
            for i, t in enumerate((t1, t2)):
                for j in range(KG):
                    nc.tensor.matmul(
                        psm[:],
                        ones[:],
                        t[:, j, :],
                        start=(i == 0 and j == 0),
                        stop=(i == 1 and j == KG - 1),
                    )
            r = sp.tile([1, CHUNK], f32, name="r", tag="r")
            nc.vector.reciprocal_approx_fast(r[:], psm[:])
            rb = sp.tile([128, CHUNK], f32, name="rb", tag="rb")
            nc.gpsimd.partition_broadcast(rb[:], r[0:1, :])

            pu = psUO.tile([128, CHUNK], f32, name="pu", tag="uo")
            for g in range(NG):
                for j in range(KG):
                    kt = g * KG + j
                    nc.tensor.matmul(
                        pu[:],
                        vTh[s][:, kt, :],
                        egs[g][:, j, :],
                        start=(kt == 0),
                        stop=(kt == KT - 1),
                    )

            un = sp.tile([128, CHUNK], f32r, name="un", tag="un")
            nc.vector.tensor_mul(un[:], pu[:], rb[:])

            for mt in range(2):
                po = psUO.tile([128, CHUNK], f32, name="po", tag="uo")
                nc.tensor.matmul(
                    po[:], wa[:, mt, :], un[:], start=True, stop=True
                )
                yt = yp.tile([128, CHUNK], bf16, name="yt", tag="y")
                nc.vector.tensor_add(yt[:], po[:], xr[:, s, mt, cs])
                nc.sync.dma_start(
                    y_d[s, mt * 128 : (mt + 1) * 128, cs], yt[:]
                )

        # ---- schedule ----
        # conv(s0) first, then attn(s0) with conv(s1)+tr(s1) thunks popped
        # between exp groups so later engines never starve; then attn(s1).
        from collections import deque

        for cc in range(NCHUNK):
            conv_qk(0, cc)
        for vc in range(NCHUNK):
            conv_v(0, vc)
        transpose_v(0, 0)
        transpose_v(0, 1)

        bg = deque()
        for cc in range(NCHUNK):
            bg.append(lambda cc=cc: conv_qk(1, cc))
        for vc in range(NCHUNK):
            bg.append(lambda vc=vc: conv_v(1, vc))

        for ck in range(NCHUNK):
            attn_chunk(0, ck, bg, per_group=1, tail=0)
        while bg:
            bg.popleft()()
        transpose_v(1, 0)
        transpose_v(1, 1)
        for ck in range(NCHUNK):
            attn_chunk(1, ck, None)

    nc.compile()
    return nc


def _get_program():
    if "nc" not in _built:
        _built["nc"] = _build_program()
    return _built["nc"]


def _make_in_maps(x, Wq, Wk, Wv, Wa, gamma):
    import ml_dtypes

    f8 = ml_dtypes.float8_e4m3
    bf = ml_dtypes.bfloat16

    x = np.asarray(x, dtype=np.float32).reshape(B, C, HWF)
    Wq = np.asarray(Wq, dtype=np.float32)
    Wk = np.asarray(Wk, dtype=np.float32)
    Wv = np.asarray(Wv, dtype=np.float32)
    Wa = np.asarray(Wa, dtype=np.float32)
    g = float(np.asarray(gamma).reshape(-1)[0])

    # x8[s, p, t, f] = x[s, t*128+p, f]
    xs = x.reshape(B, 2, 128, HWF)
    x8 = np.ascontiguousarray(xs.transpose(0, 2, 1, 3)).astype(f8)
    xr = np.ascontiguousarray(xs).astype(bf)

    # wqk8[p, t, m]: m 0-31 -> Wq[m, t*128+p], m 32-63 -> Wk[m-32, ...]
    wqk = np.empty((128, 2, 64), dtype=np.float32)
    for t in range(2):
        wqk[:, t, 0:32] = Wq[:, t * 128 : (t + 1) * 128].T
        wqk[:, t, 32:64] = Wk[:, t * 128 : (t + 1) * 128].T
    wqk8 = np.ascontiguousarray(wqk).astype(bf)

    # wv8[p, t, m] = Wv[m, t*128+p]
    wv8 = np.ascontiguousarray(
        Wv.reshape(CS, 2, 128).transpose(2, 1, 0)
    ).astype(f8)

    # waTg[p, mt, m] = g*Wa[mt*128+m, p]
    waTg = np.ascontiguousarray(
        (g * Wa).reshape(2, 128, CS).transpose(2, 0, 1)
    ).astype(np.float32)

    identb = np.eye(128, dtype=np.float32).astype(bf)
    identf = np.eye(128, dtype=np.float32)
    onesb = np.ones((128, 1), dtype=np.float32).astype(bf)

    return [
        {
            "x8": np.ascontiguousarray(x8[c * SPC : (c + 1) * SPC]),
            "xr": np.ascontiguousarray(xr[c * SPC : (c + 1) * SPC]),
            "wqk8": wqk8,
            "wv8": wv8,
            "waTg": waTg,
            "identb": identb,
            "identf": identf,
            "onesb": onesb,
        }
        for c in range(NCORES)
    ]


def kernel(x, Wq, Wk, Wv, Wa, gamma):
    from concourse import bass_utils

    nc = _get_program()
    in_maps = _make_in_maps(x, Wq, Wk, Wv, Wa, gamma)
    res = bass_utils.run_bass_kernel_spmd(
        nc, in_maps, core_ids=list(range(NCORES))
    )
    out = np.concatenate(
        [np.asarray(res.results[c]["y"]).astype(np.float32) for c in range(NCORES)],
        axis=0,
    )
    return out.reshape(B, C, H, W)
